# revision 2
# baseline (speedup 1.0000x reference)
"""Trainium2 Bass kernel for nn_CrossAttention_72275709657317  (v3).

Reference computation (B=4, S=2048, E=1024, D=64):
    Q = x @ Wq.T + bq                      [B,S,D]
    K = y @ Wk.T + bk                      [B,S,D]
    scores = Q @ K.T / sqrt(D)             [B,Sq,Sk]
    attn = softmax(scores, axis=1)         (softmax over the QUERY axis)
    V = (y @ WvR.T + bvR) @ WvL.T + bvL    [B,S,E]
    out = attn @ V                         [B,S,E]

v3 changes over v2:
  * Inputs land pre-transposed via HWDGE xbar DMA-transpose (x on the SP
    ring, y on the ACT ring) straight into xT/yT [128, 8ec, q] -- the 128
    PE transposes and ~10us of DVE PSUM-drain copies are gone.
  * den AllReduce split in two: chunks 0-7 (local K) exchange right after
    the 8th exp, so the local half of the O1 contraction runs *inside*
    the exp window; only the partner half remains in the tail.  The v2
    PE warm-up filler is gone.
  * ACT does only the 16 exps (+ tail output copies); everything else
    rides DVE/PE/SP/gpsimd.
  * O1 accumulates in two bursts (local k-chunks then partner) into one
    PSUM bank per q-half with skip_group_check.

Sharding: 8 cores -> (batch b = c//2, query-half h = c%2), as v2: each
core projects K/VR for its local k-half; pair exchanges K (f32, two
halves), VR (f32) and den partials (f32, two halves) via pairwise
AllReduce with the sum-minus-mine identity.
"""
import numpy as np

import concourse.bass as bass
import concourse.tile as tile
from concourse import bacc, mybir
from concourse.bass_utils import run_bass_kernel_spmd

N_CORES = 8
B, S, E, D = 4, 2048, 1024, 64
H = S // 2            # per-core q rows / local k rows
P = 128
EB = E // P           # 8 e-chunks
KCL = 8               # local k-chunks of 128
KC = 16               # global k-chunks
DV = D + 1            # VR width plus folded-ones column
F32 = mybir.dt.float32
BF = mybir.dt.bfloat16
EXP = mybir.ActivationFunctionType.Exp
ADD = mybir.AluOpType.add
GROUPS = [[0, 1], [2, 3], [4, 5], [6, 7]]

IN_SPECS = [
    ("x", [H, E], BF), ("y", [H, E], BF),
    ("WqT", [E, D], BF), ("WkT", [E, D], BF), ("WvRT", [E, D], BF),
    ("WvLTu", [DV, E], BF),   # rows 0:64 WvL^T, row 64 = bvL + WvL@bvR
    ("b2", [D, 2], F32),      # cols: bq, bk
]


def _emit(tc, aps, out_ap, no_cc=False, stop_stage=99):
    nc = tc.nc
    from contextlib import ExitStack
    with ExitStack() as ctx:
        const = ctx.enter_context(tc.tile_pool(name="const", bufs=1))
        big = ctx.enter_context(tc.tile_pool(name="big", bufs=1))
        outp = ctx.enter_context(tc.tile_pool(name="outp", bufs=3))
        dram = ctx.enter_context(tc.tile_pool(name="dram", bufs=1, space="DRAM"))

        # ---------------- weights: SWDGE (gpsimd) queue ----------------
        WqT_w = const.tile([P, EB, D], BF)
        nc.gpsimd.dma_start(WqT_w[:], aps["WqT"].rearrange("(c p) d -> p c d", p=P))
        WkT_w = const.tile([P, EB, D], BF)
        nc.gpsimd.dma_start(WkT_w[:], aps["WkT"].rearrange("(c p) d -> p c d", p=P))
        b2_sb = const.tile([D, 2], F32)
        nc.gpsimd.dma_start(b2_sb[:], aps["b2"])
        WvRT_w = const.tile([P, EB, D], BF)
        nc.gpsimd.dma_start(WvRT_w[:], aps["WvRT"].rearrange("(c p) d -> p c d", p=P))
        WvLT = const.tile([DV, E], BF)
        nc.gpsimd.dma_start(WvLT[:], aps["WvLTu"])

        # ---------------- persistent tiles ----------------
        xT = big.tile([P, EB, H], BF, name="xT")
        yT = big.tile([P, EB, H], BF, name="yT")
        QT = big.tile([D, H], BF, name="QT")
        KTl = big.tile([D, H], BF, name="KTl")
        KTr = big.tile([D, H], BF, name="KTr")
        blobK = big.tile([P, 512], F32, name="blobK")   # K^T f32, 2x64 rows
        blobV = big.tile([P, 512], F32, name="blobV")   # VR, 8 chunks of 64
        kvsK = big.tile([P, 512], F32, name="kvsK")
        kvsV = big.tile([P, 512], F32, name="kvsV")
        partnerK = big.tile([P, 512], F32, name="partnerK")
        partnerV = big.tile([P, 512], F32, name="partnerV")
        attnT = big.tile([P, KC, H], BF, name="attnT")
        den2 = big.tile([P, KC], F32, name="den2")
        dsum = big.tile([P, KC], F32, name="dsum")
        r_sb = big.tile([P, KC], F32, name="r_sb")
        VRp = big.tile([P, KC, P], BF, name="VRp")
        nc.gpsimd.memset(VRp[:], 0.0)
        O1T = big.tile([DV, H], BF, name="O1T")
        bias_q = b2_sb[:, 0:1]
        bias_k = b2_sb[:, 1:2]

        kvKa_dram = dram.tile([D, 512], F32)
        kvKa_sum = dram.tile([D, 512], F32)
        kvKb_dram = dram.tile([D, 512], F32)
        kvKb_sum = dram.tile([D, 512], F32)
        kvV_dram = dram.tile([P, 512], F32)
        kvV_sum = dram.tile([P, 512], F32)
        denA_dram = dram.tile([P, KCL], F32)
        denA_sum = dram.tile([P, KCL], F32)
        denB_dram = dram.tile([P, KCL], F32)
        denB_sum = dram.tile([P, KCL], F32)

        def allreduce(dst_dram, src_dram):
            if no_cc:
                nc.sync.dma_start(dst_dram[:], src_dram[:])
            else:
                nc.gpsimd.collective_compute(
                    "AllReduce", ADD, replica_groups=GROUPS,
                    ins=[src_dram.opt()], outs=[dst_dram.opt()])

        # ---------------- input DMA-transposes ----------------
        # x on the SP ring (2 x 1MB), y on the ACT ring (4 x 0.5MB); both
        # sources are fully contiguous DRAM slabs -> xbar at full rate.
        nc.sync.dma_start_transpose(xT[:, :, 0:512], aps["x"][0:512, :])
        nc.sync.dma_start_transpose(xT[:, :, 512:1024], aps["x"][512:1024, :])
        for j in range(4):
            nc.scalar.dma_start_transpose(
                yT[:, :, j * 256:(j + 1) * 256],
                aps["y"][j * 256:(j + 1) * 256, :])

        with tc.tile_pool(name="pj_ps", bufs=2, space="PSUM") as pj_ps, \
             tc.tile_pool(name="sc_ps", bufs=2, space="PSUM") as sc_ps, \
             tc.tile_pool(name="o1_ps", bufs=2, space="PSUM") as o1_ps:

            # ---------------- block-level helpers ----------------
            def q_chain(i):
                ps = pj_ps.tile([P, 256], F32, name="pj")
                for ec in range(EB):
                    nc.tensor.matmul(ps[0:D, :], WqT_w[:, ec, :],
                                     xT[:, ec, i * 256:(i + 1) * 256],
                                     start=(ec == 0), stop=(ec == EB - 1))
                nc.vector.tensor_scalar_add(QT[:, i * 256:(i + 1) * 256],
                                            ps[0:D, :], bias_q[:])

            def k_chain(i):
                ps = pj_ps.tile([P, 256], F32, name="pj")
                for ec in range(EB):
                    nc.tensor.matmul(ps[0:D, :], WkT_w[:, ec, :],
                                     yT[:, ec, i * 256:(i + 1) * 256],
                                     start=(ec == 0), stop=(ec == EB - 1))
                r0 = (i // 2) * D
                c0 = (i % 2) * 256
                nc.vector.tensor_scalar_add(blobK[r0:r0 + D, c0:c0 + 256],
                                            ps[0:D, :], bias_k[:])

            def cast_ktl(i):
                r0 = (i // 2) * D
                c0 = (i % 2) * 256
                nc.vector.tensor_copy(KTl[:, i * 256:(i + 1) * 256],
                                      blobK[r0:r0 + D, c0:c0 + 256])

            def vr_chain(kb):
                ps = pj_ps.tile([P, 256], F32, name="pj")
                for ec in range(EB):
                    nc.tensor.matmul(ps[:, 0:D], yT[:, ec, kb * P:(kb + 1) * P],
                                     WvRT_w[:, ec, :],
                                     start=(ec == 0), stop=(ec == EB - 1))
                nc.vector.tensor_copy(blobV[:, kb * D:(kb + 1) * D],
                                      ps[:, 0:D])

            def score_exp2(kcg, kt):
                sps = sc_ps.tile([P, 1024], F32, name="sc")
                kcc = kcg % 8
                for qc in range(2):
                    nc.tensor.matmul(sps[:, qc * 512:(qc + 1) * 512],
                                     kt[:, kcc * P:(kcc + 1) * P],
                                     QT[:, qc * 512:(qc + 1) * 512],
                                     start=True, stop=True)
                nc.scalar.activation(attnT[:, kcg, :], sps[:], EXP, scale=0.125,
                                     accum_out=den2[:, kcg:kcg + 1])

            # ---------------- phase 1: projections ----------------
            q_chain(0)
            q_chain(1)
            k_chain(0)
            cast_ktl(0)
            q_chain(2)
            q_chain(3)
            k_chain(1)
            cast_ktl(1)

            if stop_stage <= 1:
                nc.sync.dma_start(out_ap[0:D, 0:512], QT[:, 0:512].bitcast(BF))
                return

            # K exchange half A (k 0:512) fires as soon as k0/k1 land
            nc.sync.dma_start(kvKa_dram[:], blobK[0:D, :])
            allreduce(kvKa_sum, kvKa_dram)
            nc.sync.dma_start(kvsK[0:D, :], kvKa_sum[:])

            k_chain(2)
            cast_ktl(2)
            k_chain(3)
            cast_ktl(3)

            nc.sync.dma_start(kvKb_dram[:], blobK[D:P, :])
            allreduce(kvKb_sum, kvKb_dram)
            nc.sync.dma_start(kvsK[D:P, :], kvKb_sum[:])

            # ---------------- local scores + exps ----------------
            score_exp2(0, KTl)
            score_exp2(1, KTl)
            score_exp2(2, KTl)
            score_exp2(3, KTl)
            nc.vector.tensor_sub(partnerK[0:D, :], kvsK[0:D, :], blobK[0:D, :])
            nc.vector.tensor_copy(KTr[:, 0:512], partnerK[0:D, :])
            score_exp2(4, KTl)
            score_exp2(5, KTl)
            score_exp2(6, KTl)
            score_exp2(7, KTl)
            nc.vector.tensor_sub(partnerK[D:P, :], kvsK[D:P, :],
                                 blobK[D:P, :])
            nc.vector.tensor_copy(KTr[:, 512:1024], partnerK[D:P, :])

            if stop_stage <= 2:
                nc.sync.dma_start(out_ap[0:D, 0:512], KTr[:, 0:512].bitcast(BF))
                return

            # den exchange half A: local k-chunks 0-7, right after exp 7
            nc.sync.dma_start(denA_dram[:], den2[:, 0:KCL])
            allreduce(denA_sum, denA_dram)
            nc.sync.dma_start(dsum[:, 0:KCL], denA_sum[:])

            # ---------------- VR + its exchange (during exp window) ----
            for kb in range(8):
                vr_chain(kb)
            nc.sync.dma_start(kvV_dram[:], blobV[:])
            allreduce(kvV_sum, kvV_dram)
            nc.sync.dma_start(kvsV[:], kvV_sum[:])
            nc.vector.tensor_sub(partnerV[:], kvsV[:], blobV[:])

            # ---------------- partner scores + exps ----------------
            for kcg in range(8, 16):
                score_exp2(kcg, KTr)

            # ---------------- local O1 (inside exp window) ----------
            nc.vector.reciprocal(r_sb[:, 0:KCL], dsum[:, 0:KCL])
            nc.vector.tensor_copy(VRp[:, 0:KCL, D:DV], r_sb[:, 0:KCL])
            for c in range(KCL):
                nc.vector.tensor_scalar_mul(
                    VRp[:, c, 0:D], blobV[:, c * D:(c + 1) * D],
                    r_sb[:, c:c + 1])

            o1p = [o1_ps.tile([P, 512], F32, name="o1") for _ in range(2)]
            for qh in range(2):
                for c in range(KCL):
                    nc.tensor.matmul(o1p[qh][:], VRp[:, c, :],
                                     attnT[:, c, qh * 512:(qh + 1) * 512],
                                     start=(c == 0), stop=False,
                                     skip_group_check=True)

            # den half B (partner chunks) -- the only tail collective
            nc.sync.dma_start(denB_dram[:], den2[:, KCL:KC])
            allreduce(denB_sum, denB_dram)
            nc.sync.dma_start(dsum[:, KCL:KC], denB_sum[:])

            nc.vector.reciprocal(r_sb[:, KCL:KC], dsum[:, KCL:KC])
            nc.vector.tensor_copy(VRp[:, KCL:KC, D:DV], r_sb[:, KCL:KC])
            for c in range(KCL, KC):
                nc.vector.tensor_scalar_mul(
                    VRp[:, c, 0:D], partnerV[:, (c - KCL) * D:(c - KCL + 1) * D],
                    r_sb[:, c:c + 1])

            for qh in range(2):
                for c in range(KCL, KC):
                    nc.tensor.matmul(o1p[qh][:], VRp[:, c, :],
                                     attnT[:, c, qh * 512:(qh + 1) * 512],
                                     start=False, stop=(c == KC - 1),
                                     skip_group_check=True)
                nc.vector.tensor_copy(O1T[:, qh * 512:(qh + 1) * 512],
                                      o1p[qh][0:DV, :])

            if stop_stage <= 3:
                nc.sync.dma_start(out_ap[0:P, 0:KC], attnT[:, :, 0].bitcast(BF))
                return

        # ---------------- finals ----------------
        with tc.tile_pool(name="fin_ps", bufs=2, space="PSUM") as fin_ps:
            def final(qo):
                fps = fin_ps.tile([P, E], F32, name="fin")
                for vc in range(2):
                    nc.tensor.matmul(fps[:, vc * 512:(vc + 1) * 512],
                                     O1T[:, qo * P:(qo + 1) * P],
                                     WvLT[:, vc * 512:(vc + 1) * 512],
                                     start=True, stop=True)
                ob = outp.tile([P, E], BF, name="ob")
                nc.vector.tensor_copy(ob[:, 0:512], fps[:, 0:512])
                nc.scalar.copy(ob[:, 512:1024], fps[:, 512:1024])
                oeng = nc.sync if qo % 2 == 0 else nc.scalar
                oeng.dma_start(out_ap[qo * P:(qo + 1) * P, :], ob[:])

            for qo in range(8):
                final(qo)


def build_nc(reps: int = 1, no_cc=False, stop_stage=99):
    nc = bacc.Bacc("TRN2", target_bir_lowering=False, debug=False,
                   num_devices=N_CORES)
    aps = {name: nc.dram_tensor(name, shape, dt, kind="ExternalInput").ap()
           for name, shape, dt in IN_SPECS}
    out_ap = nc.dram_tensor("out", [H, E], BF, kind="ExternalOutput").ap()
    with tile.TileContext(nc) as tc:
        if reps == 1:
            _emit(tc, aps, out_ap, no_cc=no_cc, stop_stage=stop_stage)
        else:
            with tc.For_i(0, reps, 1):
                _emit(tc, aps, out_ap, no_cc=no_cc, stop_stage=stop_stage)
    nc.compile()
    return nc


def make_in_maps(inputs):
    import ml_dtypes
    bf = ml_dtypes.bfloat16
    arrs = {k: np.asarray(v, dtype=np.float32) for k, v in inputs.items()}
    u = arrs["bvL"] + arrs["WvL"] @ arrs["bvR"]
    wb = {
        "WqT": np.ascontiguousarray(arrs["Wq"].T.astype(bf)),
        "WkT": np.ascontiguousarray(arrs["Wk"].T.astype(bf)),
        "WvRT": np.ascontiguousarray(arrs["WvR"].T.astype(bf)),
        "WvLTu": np.ascontiguousarray(
            np.concatenate([arrs["WvL"].T, u[None, :]], axis=0).astype(bf)),
        "b2": np.ascontiguousarray(
            np.stack([arrs["bq"], arrs["bk"]], axis=1).astype(np.float32)),
    }
    xb = arrs["x"].astype(bf)
    yb = arrs["y"].astype(bf)
    in_maps = []
    for c in range(N_CORES):
        b, h = divmod(c, 2)
        m = {"x": np.ascontiguousarray(xb[b, h * H:(h + 1) * H, :]),
             "y": np.ascontiguousarray(yb[b, h * H:(h + 1) * H, :])}
        m.update(wb)
        in_maps.append(m)
    return in_maps


def assemble_out(results):
    out = np.empty((B, S, E), dtype=np.float32)
    for c in range(N_CORES):
        b, h = divmod(c, 2)
        out[b, h * H:(h + 1) * H, :] = results[c]["out"].astype(np.float32)
    return out


_NC = None


def kernel(**inputs) -> np.ndarray:
    global _NC
    if _NC is None:
        _NC = build_nc()
    in_maps = make_in_maps(inputs)
    res = run_bass_kernel_spmd(_NC, in_maps, list(range(N_CORES)))
    return assemble_out(res.results)


# revision 11
# speedup vs baseline: 1.1862x; 1.1862x over previous
"""Trainium2 Bass kernel for nn_CrossAttention_72275709657317  (v4).

Reference computation (B=4, S=2048, E=1024, D=64):
    Q = x @ Wq.T + bq                      [B,S,D]
    K = y @ Wk.T + bk                      [B,S,D]
    scores = Q @ K.T / sqrt(D)             [B,Sq,Sk]
    attn = softmax(scores, axis=1)         (softmax over the QUERY axis)
    V = (y @ WvR.T + bvR) @ WvL.T + bvL    [B,S,E]
    out = attn @ V                         [B,S,E]

v4 = v2 input path (plain streamed loads + PE transposes, which keep the
PE HAM-warm through the input phase and the DMA device at line rate) +:
  * scores row-packed: the d=64 contraction uses 64x128 PE row tiles --
    chunk pairs (p, p+4) run concurrently at tile_position (0,0)/(64,0)
    against a partition-duplicated QT2, halving score matmul time.
    K^T stays in the blobK top/bottom layout (KTl2/KTr2 [128,512] bf16),
    so the casts need no partition shift.
  * den AllReduce split in two: local chunks exchange right after the
    8th exp so the local O1 burst runs inside the exp window; only the
    partner den half remains in the tail.  v2's PE warm-up filler is
    gone.
  * O1 accumulates in two bursts per q-half into persistent PSUM with
    skip_group_check; O1T drains on DVE (ACT keeps only exps + tail
    output copies).

Sharding: 8 cores -> (batch b = c//2, query-half h = c%2); pairwise
AllReduce exchanges (K f32 x2, VR f32, den f32 x2) with sum-minus-mine.
"""
import numpy as np

import concourse.bass as bass
import concourse.tile as tile
from concourse import bacc, mybir
from concourse.masks import make_identity
from concourse.bass_utils import run_bass_kernel_spmd

N_CORES = 8
B, S, E, D = 4, 2048, 1024, 64
H = S // 2            # per-core q rows / local k rows
P = 128
EB = E // P           # 8 e-chunks
KCL = 8               # local k-chunks of 128
KC = 16               # global k-chunks
DV = D + 1            # VR width plus folded-ones column
F32 = mybir.dt.float32
BF = mybir.dt.bfloat16
EXP = mybir.ActivationFunctionType.Exp
ADD = mybir.AluOpType.add
GROUPS = [[0, 1], [2, 3], [4, 5], [6, 7]]

IN_SPECS = [
    ("x", [H, E], BF), ("y", [H, E], BF),
    ("WqT", [E, D], BF), ("WkT", [E, D], BF), ("WvRT", [E, D], BF),
    ("WvLTu", [DV, E], BF),   # rows 0:64 WvL^T, row 64 = bvL + WvL@bvR
    ("b2", [D, 2], F32),      # cols: bq, bk
]


def _emit(tc, aps, out_ap, no_cc=False, stop_stage=99):
    nc = tc.nc
    from contextlib import ExitStack
    with ExitStack() as ctx:
        const = ctx.enter_context(tc.tile_pool(name="const", bufs=1))
        io = ctx.enter_context(tc.tile_pool(name="io", bufs=8))
        big = ctx.enter_context(tc.tile_pool(name="big", bufs=1))
        outp = ctx.enter_context(tc.tile_pool(name="outp", bufs=3))
        dram = ctx.enter_context(tc.tile_pool(name="dram", bufs=1, space="DRAM"))

        # ---------------- constants / weights (SWDGE queue) -----------
        identB = const.tile([P, P], BF)
        make_identity(nc, identB[:])

        WqT_w = const.tile([P, EB, D], BF)
        nc.gpsimd.dma_start(WqT_w[:], aps["WqT"].rearrange("(c p) d -> p c d", p=P))
        WkT_w = const.tile([P, EB, D], BF)
        nc.gpsimd.dma_start(WkT_w[:], aps["WkT"].rearrange("(c p) d -> p c d", p=P))
        b2_sb = const.tile([D, 2], F32)
        nc.gpsimd.dma_start(b2_sb[:], aps["b2"])
        WvRT_w = const.tile([P, EB, D], BF)
        nc.gpsimd.dma_start(WvRT_w[:], aps["WvRT"].rearrange("(c p) d -> p c d", p=P))
        WvLT = const.tile([DV, E], BF)
        nc.gpsimd.dma_start(WvLT[:], aps["WvLTu"])

        # input loads, in stream order: x on SP queue, y on ACT queue
        inb = []
        for src, i in [("x", 0), ("y", 0), ("x", 1), ("y", 1),
                       ("x", 2), ("y", 2), ("x", 3), ("y", 3)]:
            t = io.tile([P, 2, E], BF, name="inb")
            eng = nc.sync if src == "x" else nc.scalar
            eng.dma_start(
                t[:], aps[src][i * 256:(i + 1) * 256, :]
                .rearrange("(c p) e -> p c e", p=P))
            inb.append((src, i, t))
        inb = {(s, i): t for s, i, t in inb}

        # ---------------- persistent tiles ----------------
        xT = big.tile([P, EB, H], BF, name="xT")
        yT = big.tile([P, EB, H], BF, name="yT")
        QT2 = big.tile([P, H], BF, name="QT2")          # Q^T on both halves
        KTl2 = big.tile([P, 512], BF, name="KTl2")      # local K^T, top/bot
        KTr2 = big.tile([P, 512], BF, name="KTr2")      # partner K^T
        blobK = big.tile([P, 512], F32, name="blobK")
        blobV = big.tile([P, 512], F32, name="blobV")
        kvsK = big.tile([P, 512], F32, name="kvsK")
        kvsV = big.tile([P, 512], F32, name="kvsV")
        partnerK = big.tile([P, 512], F32, name="partnerK")
        partnerV = big.tile([P, 512], F32, name="partnerV")
        attnT = big.tile([P, KC, H], BF, name="attnT")
        den2 = big.tile([P, KC], F32, name="den2")
        dsum = big.tile([P, KC], F32, name="dsum")
        r_sb = big.tile([P, KC], F32, name="r_sb")
        VRp = big.tile([P, KC, P], BF, name="VRp")
        nc.gpsimd.memset(VRp[:], 0.0)
        O1T = big.tile([DV, H], BF, name="O1T")
        bias_q = b2_sb[:, 0:1]
        bias_k = b2_sb[:, 1:2]

        kvKa_dram = dram.tile([P, 256], F32)
        kvKa_sum = dram.tile([P, 256], F32)
        kvKb_dram = dram.tile([P, 256], F32)
        kvKb_sum = dram.tile([P, 256], F32)
        kvV_dram = dram.tile([P, 512], F32)
        kvV_sum = dram.tile([P, 512], F32)
        denA_dram = dram.tile([P, KCL], F32)
        denA_sum = dram.tile([P, KCL], F32)
        denB_dram = dram.tile([P, KCL], F32)
        denB_sum = dram.tile([P, KCL], F32)

        def allreduce(dst_dram, src_dram):
            if no_cc:
                # small stand-in with latency comparable to the pipelined CC
                pr = min(64, dst_dram.shape[0])
                pc = min(64, dst_dram.shape[1])
                nc.gpsimd.dma_start(dst_dram[0:pr, 0:pc],
                                    src_dram[0:pr, 0:pc])
            else:
                nc.gpsimd.collective_compute(
                    "AllReduce", ADD, replica_groups=GROUPS,
                    ins=[src_dram.opt()], outs=[dst_dram.opt()])

        with tc.tile_pool(name="pj_ps", bufs=2, space="PSUM") as pj_ps, \
             tc.tile_pool(name="sc_ps", bufs=2, space="PSUM") as sc_ps:
            tp_ctx = tc.tile_pool(name="tp_ps", bufs=2, space="PSUM")
            tp_ps = tp_ctx.__enter__()

            # ---------------- block-level helpers ----------------
            def transpose_block(src, i, dstT, acts=(0,)):
                xb = inb[(src, i)]
                for c in range(2):
                    ps = tp_ps.tile([P, 8 * P], BF, name="tp")
                    for ec in range(EB):
                        nc.tensor.transpose(ps[:, ec * P:(ec + 1) * P],
                                            xb[:, c, ec * P:(ec + 1) * P],
                                            identB[:])
                    dst = dstT[:, :, i * 256 + c * P: i * 256 + (c + 1) * P]
                    src_ps = ps[:].rearrange("p (a b) -> p a b", a=EB)
                    if c in acts:
                        nc.scalar.copy(dst, src_ps)
                    else:
                        nc.vector.tensor_copy(dst, src_ps)

            def q_chain(i):
                ps = pj_ps.tile([P, 256], F32, name="pj")
                for ec in range(EB):
                    nc.tensor.matmul(ps[0:D, :], WqT_w[:, ec, :],
                                     xT[:, ec, i * 256:(i + 1) * 256],
                                     start=(ec == 0), stop=(ec == EB - 1))
                nc.vector.tensor_scalar_add(QT2[0:D, i * 256:(i + 1) * 256],
                                            ps[0:D, :], bias_q[:])

            def k_chain(i):
                # k-chunks 2i (top rows) and 2i+1 (bottom rows), col slot i
                ps = pj_ps.tile([P, 256], F32, name="pj")
                for ec in range(EB):
                    nc.tensor.matmul(ps[0:D, :], WkT_w[:, ec, :],
                                     yT[:, ec, i * 256:(i + 1) * 256],
                                     start=(ec == 0), stop=(ec == EB - 1))
                c0 = i * P
                nc.vector.tensor_scalar_add(blobK[0:D, c0:c0 + P],
                                            ps[0:D, 0:P], bias_k[:])
                nc.vector.tensor_scalar_add(blobK[D:P, c0:c0 + P],
                                            ps[0:D, P:2 * P], bias_k[:])

            def cast_ktl(i):
                # blobK col slot i -> KTl2 bf16, same partitions (no shift)
                c0 = i * P
                nc.vector.tensor_copy(KTl2[:, c0:c0 + P], blobK[:, c0:c0 + P])

            def vr_chain(kb):
                ps = pj_ps.tile([P, 256], F32, name="pj")
                for ec in range(EB):
                    nc.tensor.matmul(ps[:, 0:D], yT[:, ec, kb * P:(kb + 1) * P],
                                     WvRT_w[:, ec, :],
                                     start=(ec == 0), stop=(ec == EB - 1))
                nc.vector.tensor_copy(blobV[:, kb * D:(kb + 1) * D],
                                      ps[:, 0:D])

            def score_exp_pair(p, kt2, base):
                # chunk pair (base+2p, base+2p+1): 64x128 row tiles run the
                # top and bottom k-chunks concurrently.
                cT = base + 2 * p
                cB = base + 2 * p + 1
                spsT = sc_ps.tile([P, 1024], F32, name="sc")
                spsB = sc_ps.tile([P, 1024], F32, name="sc")
                for qc in range(2):
                    nc.tensor.matmul(spsT[:, qc * 512:(qc + 1) * 512],
                                     kt2[0:D, p * P:(p + 1) * P],
                                     QT2[0:D, qc * 512:(qc + 1) * 512],
                                     start=True, stop=True,
                                     tile_position=(0, 0))
                    nc.tensor.matmul(spsB[:, qc * 512:(qc + 1) * 512],
                                     kt2[D:P, p * P:(p + 1) * P],
                                     QT2[D:P, qc * 512:(qc + 1) * 512],
                                     start=True, stop=True,
                                     tile_position=(64, 0))
                nc.scalar.activation(attnT[:, cT, :], spsT[:], EXP, scale=0.125,
                                     accum_out=den2[:, cT:cT + 1])
                nc.scalar.activation(attnT[:, cB, :], spsB[:], EXP, scale=0.125,
                                     accum_out=den2[:, cB:cB + 1])

            # ---------------- streamed main phase ----------------
            transpose_block("x", 0, xT, acts=(1,))
            q_chain(0)
            transpose_block("y", 0, yT, acts=())
            k_chain(0)
            cast_ktl(0)
            transpose_block("x", 1, xT, acts=(1,))
            q_chain(1)
            transpose_block("y", 1, yT, acts=())
            k_chain(1)
            cast_ktl(1)

            if stop_stage <= 1:
                nc.sync.dma_start(out_ap[0:D, 0:512], QT2[0:D, 0:512].bitcast(BF))
                return

            # K exchange half A (col slots 0,1 = k-chunks 0..3)
            nc.sync.dma_start(kvKa_dram[:], blobK[:, 0:256])
            allreduce(kvKa_sum, kvKa_dram)
            nc.sync.dma_start(kvsK[:, 0:256], kvKa_sum[:])

            transpose_block("x", 2, xT, acts=(1,))
            q_chain(2)
            transpose_block("y", 2, yT, acts=())
            k_chain(2)
            cast_ktl(2)
            transpose_block("x", 3, xT, acts=(1,))
            q_chain(3)
            transpose_block("y", 3, yT, acts=())
            k_chain(3)
            cast_ktl(3)

            # duplicate Q^T onto partitions 64:128 for the row-tiled scores
            nc.vector.tensor_copy(QT2[D:P, :], QT2[0:D, :])
            tp_ctx.__exit__(None, None, None)   # transposes done; free banks

            # K exchange half B (col slots 2,3 = k-chunks 4..7)
            nc.sync.dma_start(kvKb_dram[:], blobK[:, 256:512])
            allreduce(kvKb_sum, kvKb_dram)
            nc.sync.dma_start(kvsK[:, 256:512], kvKb_sum[:])

            # ---------------- local scores + exps (row-packed) --------
            score_exp_pair(0, KTl2, 0)
            score_exp_pair(1, KTl2, 0)
            nc.vector.tensor_sub(partnerK[:, 0:256], kvsK[:, 0:256],
                                 blobK[:, 0:256])
            nc.vector.tensor_copy(KTr2[:, 0:256], partnerK[:, 0:256])
            score_exp_pair(2, KTl2, 0)
            score_exp_pair(3, KTl2, 0)
            nc.vector.tensor_sub(partnerK[:, 256:512], kvsK[:, 256:512],
                                 blobK[:, 256:512])
            nc.vector.tensor_copy(KTr2[:, 256:512], partnerK[:, 256:512])

            if stop_stage <= 2:
                nc.sync.dma_start(out_ap[0:D, 0:512], KTr2[0:D, :].bitcast(BF))
                return

            # den exchange half A: local k-chunks 0-7, right after exp 7
            nc.sync.dma_start(denA_dram[:], den2[:, 0:KCL])
            allreduce(denA_sum, denA_dram)
            nc.sync.dma_start(dsum[:, 0:KCL], denA_sum[:])

            # ---------------- VR + its exchange (during exp window) ---
            for kb in range(8):
                vr_chain(kb)
            nc.sync.dma_start(kvV_dram[:], blobV[:])
            allreduce(kvV_sum, kvV_dram)
            nc.sync.dma_start(kvsV[:], kvV_sum[:])
            nc.vector.tensor_sub(partnerV[:], kvsV[:], blobV[:])

            # ---------------- partner scores + exps ----------------
            for p in range(4):
                score_exp_pair(p, KTr2, 8)

            with tc.tile_pool(name="o1_ps", bufs=2, space="PSUM") as o1_ps:
                # local O1 burst (inside the exp window)
                nc.vector.reciprocal(r_sb[:, 0:KCL], dsum[:, 0:KCL])
                nc.vector.tensor_copy(VRp[:, 0:KCL, D:DV], r_sb[:, 0:KCL])
                for c in range(KCL):
                    nc.vector.tensor_scalar_mul(
                        VRp[:, c, 0:D], blobV[:, c * D:(c + 1) * D],
                        r_sb[:, c:c + 1])

                o1p = [o1_ps.tile([P, 512], F32, name="o1") for _ in range(2)]
                for qh in range(2):
                    for c in range(KCL):
                        nc.tensor.matmul(o1p[qh][:], VRp[:, c, :],
                                         attnT[:, c, qh * 512:(qh + 1) * 512],
                                         start=(c == 0), stop=False,
                                         skip_group_check=True)

                # den half B (partner chunks) -- the only tail collective
                nc.sync.dma_start(denB_dram[:], den2[:, KCL:KC])
                allreduce(denB_sum, denB_dram)
                nc.sync.dma_start(dsum[:, KCL:KC], denB_sum[:])

                nc.vector.reciprocal(r_sb[:, KCL:KC], dsum[:, KCL:KC])
                nc.vector.tensor_copy(VRp[:, KCL:KC, D:DV], r_sb[:, KCL:KC])
                for c in range(KCL, KC):
                    nc.vector.tensor_scalar_mul(
                        VRp[:, c, 0:D],
                        partnerV[:, (c - KCL) * D:(c - KCL + 1) * D],
                        r_sb[:, c:c + 1])

                for qh in range(2):
                    for c in range(KCL, KC):
                        nc.tensor.matmul(o1p[qh][:], VRp[:, c, :],
                                         attnT[:, c, qh * 512:(qh + 1) * 512],
                                         start=False, stop=(c == KC - 1),
                                         skip_group_check=True)
                    nc.vector.tensor_copy(O1T[:, qh * 512:(qh + 1) * 512],
                                          o1p[qh][0:DV, :])

                if stop_stage <= 3:
                    nc.sync.dma_start(out_ap[0:P, 0:KC],
                                      attnT[:, :, 0].bitcast(BF))
                    return

        # ---------------- finals ----------------
        with tc.tile_pool(name="fin_ps", bufs=3, space="PSUM") as fin_ps:
            def final(qo):
                fps = fin_ps.tile([P, E], F32, name="fin")
                for vc in range(2):
                    nc.tensor.matmul(fps[:, vc * 512:(vc + 1) * 512],
                                     O1T[:, qo * P:(qo + 1) * P],
                                     WvLT[:, vc * 512:(vc + 1) * 512],
                                     start=True, stop=True)
                ob = outp.tile([P, E], BF, name="ob")
                nc.vector.tensor_copy(ob[:, 0:512], fps[:, 0:512])
                nc.scalar.copy(ob[:, 512:1024], fps[:, 512:1024])
                oeng = nc.sync if qo % 2 == 0 else nc.scalar
                oeng.dma_start(out_ap[qo * P:(qo + 1) * P, :], ob[:])

            for qo in range(8):
                final(qo)


def build_nc(reps: int = 1, no_cc=False, stop_stage=99):
    nc = bacc.Bacc("TRN2", target_bir_lowering=False, debug=False,
                   num_devices=N_CORES)
    aps = {name: nc.dram_tensor(name, shape, dt, kind="ExternalInput").ap()
           for name, shape, dt in IN_SPECS}
    out_ap = nc.dram_tensor("out", [H, E], BF, kind="ExternalOutput").ap()
    with tile.TileContext(nc) as tc:
        if reps == 1:
            _emit(tc, aps, out_ap, no_cc=no_cc, stop_stage=stop_stage)
        else:
            with tc.For_i(0, reps, 1):
                _emit(tc, aps, out_ap, no_cc=no_cc, stop_stage=stop_stage)
    nc.compile()
    return nc


def make_in_maps(inputs):
    import ml_dtypes
    bf = ml_dtypes.bfloat16
    arrs = {k: np.asarray(v, dtype=np.float32) for k, v in inputs.items()}
    u = arrs["bvL"] + arrs["WvL"] @ arrs["bvR"]
    wb = {
        "WqT": np.ascontiguousarray(arrs["Wq"].T.astype(bf)),
        "WkT": np.ascontiguousarray(arrs["Wk"].T.astype(bf)),
        "WvRT": np.ascontiguousarray(arrs["WvR"].T.astype(bf)),
        "WvLTu": np.ascontiguousarray(
            np.concatenate([arrs["WvL"].T, u[None, :]], axis=0).astype(bf)),
        "b2": np.ascontiguousarray(
            np.stack([arrs["bq"], arrs["bk"]], axis=1).astype(np.float32)),
    }
    xb = arrs["x"].astype(bf)
    yb = arrs["y"].astype(bf)
    in_maps = []
    for c in range(N_CORES):
        b, h = divmod(c, 2)
        m = {"x": np.ascontiguousarray(xb[b, h * H:(h + 1) * H, :]),
             "y": np.ascontiguousarray(yb[b, h * H:(h + 1) * H, :])}
        m.update(wb)
        in_maps.append(m)
    return in_maps


def assemble_out(results):
    out = np.empty((B, S, E), dtype=np.float32)
    for c in range(N_CORES):
        b, h = divmod(c, 2)
        out[b, h * H:(h + 1) * H, :] = results[c]["out"].astype(np.float32)
    return out


_NC = None


def kernel(**inputs) -> np.ndarray:
    global _NC
    if _NC is None:
        _NC = build_nc()
    in_maps = make_in_maps(inputs)
    res = run_bass_kernel_spmd(_NC, in_maps, list(range(N_CORES)))
    return assemble_out(res.results)
